# revision 60
# baseline (speedup 1.0000x reference)
"""Fused pairwise-MLP kernel for Trainium2 (8 NeuronCores, SPMD data-parallel).

Computes log_q[i, j] = W3 @ gelu(W2 @ gelu(a[j] + b[i] + b1) + b2) + b3
with a = z1 @ W1a.T, b = z2 @ W1b.T  (W1 = [W1a | W1b]), N=1024, H=EMB=128.

Sharding: rows of i (z2) split across 8 cores, z1 + weights replicated
(host-side sharding; no collectives). The [N, N, H] intermediates are never
materialized in HBM — everything stays in SBUF/PSUM per 128-row i-tile.

The host only relays/relabels inputs (transposes, sharding, zero-padding W3);
all math runs on device. Per core and per i: one 1024-wide gelu on ACT
(bias precomputed by DVE so gelu1 runs as bias-free 4096-wide quads), fp16
W2 matmuls into a manually-sliced 8-bank PSUM ring consumed by 2048-wide
gelu2 pairs, and the W3 dot as fp16 M=32 column-tile matmuls written back
into the consumed ring slot. The kernel is ACT-bound: 2 gelu passes over
16.8M elements per core at 1 elem/lane/cycle @ 1.2 GHz.
"""

import numpy as np

import concourse.bacc as bacc
import concourse.bass as bass
import concourse.tile as tile
import concourse.mybir as mybir
from concourse import bass_utils



N = 1024
EMB = 128
HID = 128
NCORES = 8
SH = N // NCORES  # i-rows per core
F32 = mybir.dt.float32
FP16 = mybir.dt.float16
GELU = mybir.ActivationFunctionType.Gelu


def _build():
    nc = bacc.Bacc("TRN2", target_bir_lowering=False, debug=False)

    z1Tq_d = [
        nc.dram_tensor(f"z1Tq{q}", (EMB, 256), F32, kind="ExternalInput")
        for q in range(4)
    ]
    z2T_d = nc.dram_tensor("z2T", (EMB, SH), F32, kind="ExternalInput")
    w1aT_d = nc.dram_tensor("w1aT", (EMB, HID), F32, kind="ExternalInput")
    w1bT_d = nc.dram_tensor("w1bT", (EMB, HID), F32, kind="ExternalInput")
    w2T_d = nc.dram_tensor("w2T", (HID, HID), F32, kind="ExternalInput")
    w3p_d = nc.dram_tensor("w3p", (HID, 32), F32, kind="ExternalInput")
    b1_d = nc.dram_tensor("b1", (HID,), F32, kind="ExternalInput")
    b2_d = nc.dram_tensor("b2", (HID,), F32, kind="ExternalInput")
    b3_d = nc.dram_tensor("b3", (1,), F32, kind="ExternalInput")
    out_d = nc.dram_tensor("out", (SH, N), F32, kind="ExternalOutput")

    with tile.TileContext(nc) as tc:
        _body(
            tc, out_d, z1Tq_d, z2T_d, w1aT_d, w1bT_d, w2T_d, w3p_d,
            b1_d, b2_d, b3_d,
        )

    nc.compile()
    return nc


def _body(tc, out_d, z1Tq_d, z2T_d, w1aT_d, w1bT_d, w2T_d, w3p_d, b1_d, b2_d, b3_d):
    nc = tc.nc
    with (
        tc.tile_pool(name="const", bufs=1) as const,
        tc.tile_pool(name="h1p", bufs=4) as h1p,
        tc.tile_pool(name="h2p", bufs=8) as h2p,
        tc.tile_pool(name="srows", bufs=8) as srows,
        tc.tile_pool(name="ringp", bufs=1, space="PSUM") as ringp,
    ):
        # ---- load inputs: z1T fans out across the sync+scalar HWDGE queues,
        # small tensors ride the gpsimd SWDGE queue ----
        w1aT_sb = const.tile([128, HID], F32)
        nc.sync.dma_start(out=w1aT_sb, in_=w1aT_d.ap())
        z1T_sb = const.tile([128, N], F32)
        nc.sync.dma_start(out=z1T_sb[:, 0:256], in_=z1Tq_d[0].ap())
        nc.scalar.dma_start(out=z1T_sb[:, 256:512], in_=z1Tq_d[1].ap())
        z2T_sb = const.tile([128, SH], F32)
        nc.sync.dma_start(out=z2T_sb, in_=z2T_d.ap())
        w1bT_sb = const.tile([128, HID], F32)
        nc.scalar.dma_start(out=w1bT_sb, in_=w1bT_d.ap())
        nc.sync.dma_start(out=z1T_sb[:, 512:768], in_=z1Tq_d[2].ap())
        nc.scalar.dma_start(out=z1T_sb[:, 768:1024], in_=z1Tq_d[3].ap())
        w2T_f = const.tile([128, HID], F32)
        nc.scalar.dma_start(out=w2T_f, in_=w2T_d.ap())
        w3p_f = const.tile([128, 32], F32)
        nc.gpsimd.dma_start(out=w3p_f, in_=w3p_d.ap())
        b1_sb = const.tile([128, 1], F32)
        nc.gpsimd.dma_start(out=b1_sb, in_=b1_d.ap().rearrange("(p o) -> p o", o=1))
        b2_sb = const.tile([128, 1], F32)
        nc.gpsimd.dma_start(out=b2_sb, in_=b2_d.ap().rearrange("(p o) -> p o", o=1))
        b3_sb = const.tile([128, 1], F32)
        nc.gpsimd.dma_start(
            out=b3_sb,
            in_=bass.AP(tensor=b3_d, offset=0, ap=[[0, 128], [1, 1]]),
        )

        # Dummy 1-element gelu so the ~2.7us ACT table load for the gelu set
        # runs right away, off the critical path of the first real gelu.
        tiny = const.tile([1, 1], F32)
        nc.vector.memset(tiny, 0.0)
        warm = const.tile([1, 1], F32)
        nc.scalar.activation(warm, tiny, GELU)

        # fp16 lhsT casts (DVE rounds on write)
        w2T_sb = const.tile([128, HID], FP16)
        nc.vector.tensor_copy(w2T_sb, w2T_f)
        w3T_sb = const.tile([128, 32], FP16)
        nc.vector.tensor_copy(w3T_sb, w3p_f)

        # ---- single manually-sliced PSUM ring (all 8 banks) ----
        # Two 2048-col pair-slots for the steady-state loop; during prep the
        # second slot holds a (cols 2048:3072) and b (3072:3200). Tile's
        # subtile dependency tracking orders all accesses.
        ring = ringp.tile([128, 4096], F32)

        # ---- a[h, j] for all j; b_pp = b + b1 ----
        # The b matmul is interleaved right after the first a-quarter so it
        # doesn't gate b_pp (and thus the first gelu) from the end of the PE
        # stream.
        tpa = ring[:, 2048:3072]
        tpb = ring[:, 3072 : 3072 + SH]
        b_pp_sb = const.tile([128, SH], F32)
        for q in range(4):
            nc.tensor.matmul(
                tpa[:, q * 256 : (q + 1) * 256],
                w1aT_sb,
                z1T_sb[:, q * 256 : (q + 1) * 256],
            )
            if q == 0:
                nc.tensor.matmul(tpb, w1bT_sb, z2T_sb)
                nc.vector.tensor_scalar_add(b_pp_sb, tpb, b1_sb[:, 0:1])

        # a stays in PSUM for the head quads' gelus; the SBUF copy (for the
        # steady-state DVE pre-adds) happens off the critical path.
        a_sb = const.tile([128, N], F32)
        nc.vector.tensor_copy(a_sb, tpa)

        # ---- main loop over my 128 i values ----
        # gelu1 runs as bias-free 4096-wide quads (DVE precomputes
        # a + b_pp[:, i] into `pre` with 2x-mode SBUF adds), emitted 2 quads
        # ahead of consumption.
        NQ = SH // 4
        h1qs = [None] * NQ

        def emit_g1_quad(q):
            h1q = h1p.tile([128, 4 * N], FP16, tag="h1q", name="h1q", bufs=3)
            if q < 2:
                # Head quads: per-i gelu with the ACT bias port, streaming a
                # straight from PSUM — skips the DVE pre-add chain and the
                # a->SBUF copy on the kernel's critical path.
                for k in range(4):
                    i = 4 * q + k
                    nc.scalar.activation(
                        h1q[:, k * N : (k + 1) * N],
                        tpa,
                        GELU,
                        bias=b_pp_sb[:, i : i + 1],
                    )
            else:
                pre = h1p.tile([128, 4 * N], F32, tag="pre", name="pre", bufs=3)
                for k in range(4):
                    i = 4 * q + k
                    nc.vector.tensor_scalar_add(
                        pre[:, k * N : (k + 1) * N], a_sb, b_pp_sb[:, i : i + 1]
                    )
                nc.scalar.activation(h1q, pre, GELU)
            h1qs[q] = h1q

        emit_g1_quad(0)
        emit_g1_quad(1)
        # Steady state runs per PAIR of i's over the two 2048-col ring
        # slots: 4 W2 matmuls fill a slot, one 2048-wide gelu2 consumes it,
        # then the W3-dot column-tile matmuls (col-group 0 for i0, group 2
        # for i1, halves serial within a group so each row lands contiguous
        # at partition 0 / 64) write back into the consumed slot's first
        # 1024 cols; one full-width DVE op evacuates them (+b3) and two row
        # DMAs write out.
        for p in range(SH // 2):
            q = p // 2
            if p % 2 == 0:
                if q + 2 < NQ:
                    emit_g1_quad(q + 2)
                h1q = h1qs[q]
                h1qs[q] = None
            s = (p % 2) * 2048
            k0 = (p % 2) * 2
            for k in range(2):
                base = (k0 + k) * N
                nc.tensor.matmul(
                    ring[:, s + k * N : s + k * N + 512],
                    w2T_sb,
                    h1q[:, base : base + 512],
                )
                nc.tensor.matmul(
                    ring[:, s + k * N + 512 : s + (k + 1) * N],
                    w2T_sb,
                    h1q[:, base + 512 : base + N],
                )

            h2pair = h2p.tile([128, 2 * N], FP16, tag="h2", bufs=4)
            nc.scalar.activation(h2pair, ring[:, s : s + 2048], GELU, bias=b2_sb[:, 0:1])

            for ii in range(2):
                for h in range(2):
                    nc.tensor.matmul(
                        ring[64 * ii : 64 * ii + 32, s + h * 512 : s + (h + 1) * 512],
                        w3T_sb,
                        h2pair[:, ii * N + h * 512 : ii * N + (h + 1) * 512],
                        tile_position=(0, 64 * ii),
                    )
            srow = srows.tile([128, N], F32, tag="srow", bufs=4)
            nc.vector.tensor_scalar_add(srow, ring[:, s : s + N], b3_sb[:, 0:1])
            for ii in range(2):
                nc.sync.dma_start(
                    out=out_d.ap()[2 * p + ii : 2 * p + ii + 1, :],
                    in_=srow[64 * ii : 64 * ii + 1, :],
                )


_NC_CACHE = None


def make_in_maps(z1, z2, W1, b1, W2, b2, W3, b3):
    f = np.float32
    z1 = np.asarray(z1, dtype=f)
    z2 = np.asarray(z2, dtype=f)
    W1 = np.asarray(W1, dtype=f)
    b1 = np.ascontiguousarray(np.asarray(b1, dtype=f))
    W2 = np.asarray(W2, dtype=f)
    b2 = np.ascontiguousarray(np.asarray(b2, dtype=f))
    W3 = np.asarray(W3, dtype=f)
    b3 = np.ascontiguousarray(np.asarray(b3, dtype=f))

    # Host-side relayout only (no math): transposes, the i-shard split of
    # z2, and zero-padding W3 to an M=32 column tile.
    z1T = np.ascontiguousarray(z1.T)
    z1Tq = {
        f"z1Tq{q}": np.ascontiguousarray(z1T[:, q * 256 : (q + 1) * 256])
        for q in range(4)
    }
    w1aT = np.ascontiguousarray(W1[:, :EMB].T)
    w1bT = np.ascontiguousarray(W1[:, EMB:].T)
    w2T = np.ascontiguousarray(W2.T)
    w3p = np.zeros((HID, 32), dtype=f)
    w3p[:, 0] = W3[0]

    return [
        {
            **z1Tq,
            "z2T": np.ascontiguousarray(z2[c * SH : (c + 1) * SH].T),
            "w1aT": w1aT,
            "w1bT": w1bT,
            "w2T": w2T,
            "w3p": w3p,
            "b1": b1,
            "b2": b2,
            "b3": b3,
        }
        for c in range(NCORES)
    ]


def kernel(z1, z2, W1, b1, W2, b2, W3, b3):
    global _NC_CACHE
    if _NC_CACHE is None:
        _NC_CACHE = _build()
    nc = _NC_CACHE

    in_maps = make_in_maps(z1, z2, W1, b1, W2, b2, W3, b3)
    res = bass_utils.run_bass_kernel_spmd(nc, in_maps, core_ids=list(range(NCORES)))
    return np.concatenate([r["out"] for r in res.results], axis=0)


if __name__ == "__main__":
    rng = np.random.default_rng(0)
    s1 = 1.0 / np.sqrt(2 * EMB)
    s2 = 1.0 / np.sqrt(HID)
    ins = dict(
        z1=rng.standard_normal((N, EMB), dtype=np.float32),
        z2=rng.standard_normal((N, EMB), dtype=np.float32),
        W1=rng.uniform(-s1, s1, (HID, 2 * EMB)).astype(np.float32),
        b1=rng.uniform(-s1, s1, (HID,)).astype(np.float32),
        W2=rng.uniform(-s2, s2, (HID, HID)).astype(np.float32),
        b2=rng.uniform(-s2, s2, (HID,)).astype(np.float32),
        W3=rng.uniform(-s2, s2, (1, HID)).astype(np.float32),
        b3=rng.uniform(-s2, s2, (1,)).astype(np.float32),
    )
    out = kernel(**ins)
    print("out", out.shape, out.dtype, out[:2, :4])
